# revision 19
# baseline (speedup 1.0000x reference)
"""Trainium2 Bass kernel for nn_ConstraintLayer (batched equality-constrained QP layer).

Math: the reference solves  M @ sol_i = [2*y_i; b_i]  for every batch row i,
with the SAME KKT matrix M = [[2I, A^T], [A, 0]] (80x80).  Since M is fixed,
    y_star = [2y, b] @ (M^{-1}[:64, :])^T  =  y @ Gy + b @ Gb
with Gy = 2*Minv[:64,:64].T (64x64) and Gb = Minv[:64,64:].T (16x64),
i.e. one skinny (batch,80)@(80,64) matmul — memory bound.

Distribution: pure data parallelism; the batch (1048576 rows) is split into 8
shards of 131072 rows, one per NeuronCore.  The tiny Gy/Gb factors are
precomputed once on host (float64 inverse of the 80x80 block matrix) and
replicated to every core.

Precision: the correctness gate is rel_err < 2e-2 (max-abs normalized), so a
single fp16 term per operand suffices: fp16(y) @ fp16(Gy) + fp16(b) @ fp16(Gb)
with fp32 PSUM accumulation, output stored fp16, measures 7.6e-4 against the
reference.  This halves HBM traffic vs the earlier fp32-exact hi/lo-split
kernel (37MB vs 78MB per core).

Device layout (per core): the host interleaves y and b into ONE feature-major
stream: per 512-row chunk a [80, 512] tile (partitions 0:64 = y features,
64:80 = b features), 8 chunks per block -> [80, 4096] 640KB contiguous DMAs.
Each chunk is a SINGLE K=80 matmul against the fused stationary [Gy; Gb]
(loaded once for the whole kernel - no LDWEIGHTS swaps, which kept the
earlier 2-matmul variant pinned at the PE's throttled 1.2 GHz rate), M=64
out partitions, N=512.  PSUM eviction (f32 -> f16 cast) alternates between
VectorE and the scalar ACT engine so neither becomes the critical path;
outputs leave as contiguous 512KB fp16 DMAs and the host inverts the packing.
"""

import numpy as np

BATCH = 1048576
IN_DIM = 64
OUT_DIM = 16
N_CORES = 8
SHARD = BATCH // N_CORES        # 131072
CHUNK = 512                     # batch rows per matmul (one PSUM bank col-span)
CPB = 8                         # chunks per block
N_BLK = SHARD // (CHUNK * CPB)  # 32
COLS = CHUNK * CPB              # 4096

_prog_cache = {}
last_results = None             # BassKernelResults of the most recent run (for test harness)


def _build_weights(A):
    """Host precompute of the fused stationary matrix (float64 inverse, fp16)."""
    m, n = A.shape  # (16, 64)
    A64 = np.asarray(A, dtype=np.float64)
    M = np.zeros((n + m, n + m))
    M[:n, :n] = 2.0 * np.eye(n)
    M[:n, n:] = A64.T
    M[n:, :n] = A64
    Minv = np.linalg.inv(M)
    W = np.zeros((128, 128), np.float16)
    W[:64, :64] = (2.0 * Minv[:n, :n].T).astype(np.float16)     # Gy
    W[96:112, :64] = (Minv[:n, n:].T).astype(np.float16)        # Gb
    # Full K=128 / M=128 shape: the PE's activity monitor only releases the
    # 1.2 -> 2.4 GHz clock throttle for (near-)full-array matmuls -- K=80,
    # K=96 and mixed K=128/32 kernels all measured 0/256 warm matmuls while
    # the K~=120-avg baseline got 375/512 warm.  Rows 80:128 are zero
    # weights; the matching moving rows are zeroed once per pool buffer.
    # M is duplicated ([W | W]); out partitions 64:128 are never read.
    W[:, 64:] = W[:, :64]
    return W


def _pack_in(ys, bs):
    # y (131072, 64) f16 + b (131072, 16) f16 -> blocks (32, 80, 4096);
    # partition = feature (y 0:64, b 64:80), col = 512*chunk_in_blk + s
    yb = ys.reshape(N_BLK, CPB, CHUNK, 64).transpose(0, 3, 1, 2).reshape(N_BLK, 64, COLS)
    bb = bs.reshape(N_BLK, CPB, CHUNK, 16).transpose(0, 3, 1, 2).reshape(N_BLK, 16, COLS)
    return np.ascontiguousarray(np.concatenate([yb, bb], axis=1))


def _unpack_out(ob):
    # (32, 64, 4096) f16 -> (131072, 64)
    return np.ascontiguousarray(
        ob.reshape(N_BLK, 64, CPB, CHUNK).transpose(0, 2, 3, 1)
    ).reshape(SHARD, 64)


def _build_program():
    import concourse.bacc as bacc
    import concourse.mybir as mybir
    import concourse.tile as tile

    f16 = mybir.dt.float16
    f32 = mybir.dt.float32
    nc = bacc.Bacc("TRN2")
    YB_d = nc.dram_tensor("YB", (N_BLK, 80, COLS), f16, kind="ExternalInput")
    W_d = nc.dram_tensor("W", (128, 128), f16, kind="ExternalInput")
    Ot = nc.dram_tensor("Ot", (N_BLK, 64, COLS), f16, kind="ExternalOutput")

    with tile.TileContext(nc) as tc:
        with (
            tc.tile_pool(name="wpool", bufs=1) as wpool,
            tc.tile_pool(name="ystat", bufs=1) as ystat,
            tc.tile_pool(name="opool", bufs=4) as opool,
            tc.tile_pool(name="pspool", bufs=8, space="PSUM") as pspool,
        ):
            # Inputs go through the sync-engine HWDGE ring, the (tiny) weight
            # through the scalar ring, outputs through SWDGE — independent
            # FIFOs so an output store never queues behind input traffic.
            # The scalar ring's first packet leaves ~6us before the sync
            # ring's (measured), so the weight AND block 0 go on it, block 0
            # in quarter-block pieces so the first matmul only waits for the
            # first 160KB instead of the whole 640KB block.
            w = wpool.tile([128, 128], f16)
            nc.scalar.dma_start(w[:], W_d[:])

            # Eight static [128, COLS] input tiles, rotated manually.  The y
            # features land on rows 0:64 and the b features on rows 96:112,
            # so the zero-pad rows (64:96, 112:128) are DISJOINT from every
            # DMA write: they are zeroed once here and never serialize with
            # the input stream.  Spread across vector+gpsimd so the first
            # tile is ready ~4us in and the rest hide behind the pipeline.
            yts = [ystat.tile([128, COLS], f16, name=f"yt{i_}")
                   for i_ in range(8)]
            for pt in yts:
                nc.vector.memset(pt[64:128, :], 0.0)

            for blk in range(N_BLK):
                yt = yts[blk % 8]
                if blk == 0:
                    q = COLS // 4
                    for pc in range(4):
                        nc.sync.dma_start(yt[0:64, pc * q:(pc + 1) * q],
                                          YB_d[0, 0:64, pc * q:(pc + 1) * q])
                    nc.sync.dma_start(yt[96:112, :], YB_d[0, 64:80, :])
                else:
                    ring = nc.sync if blk % 2 == 0 else nc.scalar
                    ring.dma_start(yt[0:64, :], YB_d[blk, 0:64, :])
                    ring.dma_start(yt[96:112, :], YB_d[blk, 64:80, :])
                otile = opool.tile([64, COLS], f16)
                for ci in range(CPB):
                    cols = slice(CHUNK * ci, CHUNK * (ci + 1))
                    ps = pspool.tile([128, CHUNK], f32)
                    nc.tensor.matmul(ps[:], w[:], yt[:, cols],
                                     start=True, stop=True)   # K=80 fused y+b
                    if ci % 2 == 0:
                        nc.vector.tensor_copy(otile[:, cols], ps[0:64, :])
                    else:
                        nc.scalar.activation(otile[:, cols], ps[0:64, :],
                                             mybir.ActivationFunctionType.Copy)
                    if ci == CPB // 2 - 1:
                        # store the first half-block early: smooths the SWDGE
                        # queue and shortens the drain tail
                        nc.gpsimd.dma_start(Ot[blk, :, :COLS // 2],
                                            otile[:, :COLS // 2])
                nc.gpsimd.dma_start(Ot[blk, :, COLS // 2:], otile[:, COLS // 2:])
    nc.compile()  # bacc passes: split sync waits to HW limits, alloc regs, DCE
    return nc


def _get_program():
    if "nc" not in _prog_cache:
        _prog_cache["nc"] = _build_program()
    return _prog_cache["nc"]


def kernel(y, A, b):
    global last_results
    from concourse.bass_utils import run_bass_kernel_spmd

    y = np.ascontiguousarray(np.asarray(y, dtype=np.float32))
    b = np.ascontiguousarray(np.asarray(b, dtype=np.float32))
    A = np.asarray(A, dtype=np.float32)
    assert y.shape == (BATCH, IN_DIM) and b.shape == (BATCH, OUT_DIM)

    W = _build_weights(A)
    yh = y.astype(np.float16)
    bh = b.astype(np.float16)

    in_maps = []
    for core in range(N_CORES):
        sl = slice(core * SHARD, (core + 1) * SHARD)
        in_maps.append({"YB": _pack_in(yh[sl], bh[sl]), "W": W})

    nc = _get_program()
    res = run_bass_kernel_spmd(nc, in_maps, core_ids=list(range(N_CORES)))
    last_results = res

    out = np.empty((BATCH, IN_DIM), np.float32)
    for core in range(N_CORES):
        out[core * SHARD:(core + 1) * SHARD] = \
            _unpack_out(res.results[core]["Ot"]).astype(np.float32)
    return out


# revision 20
# speedup vs baseline: 1.0969x; 1.0969x over previous
"""Trainium2 Bass kernel for nn_ConstraintLayer (batched equality-constrained QP layer).

Math: the reference solves  M @ sol_i = [2*y_i; b_i]  for every batch row i,
with the SAME KKT matrix M = [[2I, A^T], [A, 0]] (80x80).  Since M is fixed,
    y_star = [2y, b] @ (M^{-1}[:64, :])^T  =  y @ Gy + b @ Gb
with Gy = 2*Minv[:64,:64].T (64x64) and Gb = Minv[:64,64:].T (16x64),
i.e. one skinny (batch,80)@(80,64) matmul — memory bound.

Distribution: pure data parallelism; the batch (1048576 rows) is split into 8
shards of 131072 rows, one per NeuronCore.  The tiny Gy/Gb factors are
precomputed once on host (float64 inverse of the 80x80 block matrix) and
replicated to every core.

Precision: the correctness gate is rel_err < 2e-2 (max-abs normalized), so a
single fp16 term per operand suffices: fp16(y) @ fp16(Gy) + fp16(b) @ fp16(Gb)
with fp32 PSUM accumulation, output stored fp16, measures 7.6e-4 against the
reference.  This halves HBM traffic vs the earlier fp32-exact hi/lo-split
kernel (37MB vs 78MB per core).

Device layout (per core): the host interleaves y and b into ONE feature-major
stream: per 512-row chunk a [80, 512] tile (partitions 0:64 = y features,
64:80 = b features), 8 chunks per block -> [80, 4096] 640KB contiguous DMAs.
Each chunk is a SINGLE K=80 matmul against the fused stationary [Gy; Gb]
(loaded once for the whole kernel - no LDWEIGHTS swaps, which kept the
earlier 2-matmul variant pinned at the PE's throttled 1.2 GHz rate), M=64
out partitions, N=512.  PSUM eviction (f32 -> f16 cast) alternates between
VectorE and the scalar ACT engine so neither becomes the critical path;
outputs leave as contiguous 512KB fp16 DMAs and the host inverts the packing.
"""

import numpy as np

BATCH = 1048576
IN_DIM = 64
OUT_DIM = 16
N_CORES = 8
SHARD = BATCH // N_CORES        # 131072
CHUNK = 512                     # batch rows per matmul (one PSUM bank col-span)
CPB = 8                         # chunks per block
N_BLK = SHARD // (CHUNK * CPB)  # 32
COLS = CHUNK * CPB              # 4096

_prog_cache = {}
last_results = None             # BassKernelResults of the most recent run (for test harness)


def _build_weights(A):
    """Host precompute of the fused stationary matrix (float64 inverse, fp16)."""
    m, n = A.shape  # (16, 64)
    A64 = np.asarray(A, dtype=np.float64)
    M = np.zeros((n + m, n + m))
    M[:n, :n] = 2.0 * np.eye(n)
    M[:n, n:] = A64.T
    M[n:, :n] = A64
    Minv = np.linalg.inv(M)
    W = np.zeros((128, 128), np.float16)
    W[:64, :64] = (2.0 * Minv[:n, :n].T).astype(np.float16)     # Gy
    W[96:112, :64] = (Minv[:n, n:].T).astype(np.float16)        # Gb
    # Full K=128 / M=128 shape: the PE's activity monitor only releases the
    # 1.2 -> 2.4 GHz clock throttle for (near-)full-array matmuls -- K=80,
    # K=96 and mixed K=128/32 kernels all measured 0/256 warm matmuls while
    # the K~=120-avg baseline got 375/512 warm.  Rows 80:128 are zero
    # weights; the matching moving rows are zeroed once per pool buffer.
    # M is duplicated ([W | W]); out partitions 64:128 are never read.
    W[:, 64:] = W[:, :64]
    return W


def _pack_in(ys, bs):
    # y (131072, 64) f16 + b (131072, 16) f16 -> blocks (32, 80, 4096);
    # partition = feature (y 0:64, b 64:80), col = 512*chunk_in_blk + s
    yb = ys.reshape(N_BLK, CPB, CHUNK, 64).transpose(0, 3, 1, 2).reshape(N_BLK, 64, COLS)
    bb = bs.reshape(N_BLK, CPB, CHUNK, 16).transpose(0, 3, 1, 2).reshape(N_BLK, 16, COLS)
    return np.ascontiguousarray(np.concatenate([yb, bb], axis=1))


def _unpack_out(ob):
    # (32, 64, 4096) f16 -> (131072, 64)
    return np.ascontiguousarray(
        ob.reshape(N_BLK, 64, CPB, CHUNK).transpose(0, 2, 3, 1)
    ).reshape(SHARD, 64)


def _build_program():
    import concourse.bacc as bacc
    import concourse.mybir as mybir
    import concourse.tile as tile

    f16 = mybir.dt.float16
    f32 = mybir.dt.float32
    nc = bacc.Bacc("TRN2")
    YB_d = nc.dram_tensor("YB", (N_BLK, 80, COLS), f16, kind="ExternalInput")
    W_d = nc.dram_tensor("W", (128, 128), f16, kind="ExternalInput")
    Ot = nc.dram_tensor("Ot", (N_BLK, 64, COLS), f16, kind="ExternalOutput")

    with tile.TileContext(nc) as tc:
        with (
            tc.tile_pool(name="wpool", bufs=1) as wpool,
            tc.tile_pool(name="ystat", bufs=1) as ystat,
            tc.tile_pool(name="opool", bufs=4) as opool,
            tc.tile_pool(name="pspool", bufs=8, space="PSUM") as pspool,
        ):
            # Inputs go through the sync-engine HWDGE ring, the (tiny) weight
            # through the scalar ring, outputs through SWDGE — independent
            # FIFOs so an output store never queues behind input traffic.
            # The scalar ring's first packet leaves ~6us before the sync
            # ring's (measured), so the weight AND block 0 go on it, block 0
            # in quarter-block pieces so the first matmul only waits for the
            # first 160KB instead of the whole 640KB block.
            w = wpool.tile([128, 128], f16)
            nc.scalar.dma_start(w[:], W_d[:])

            # Eight static [128, COLS] input tiles, rotated manually.  The y
            # features land on rows 0:64 and the b features on rows 96:112,
            # so the zero-pad rows (64:96, 112:128) are DISJOINT from every
            # DMA write: they are zeroed once here and never serialize with
            # the input stream.  Spread across vector+gpsimd so the first
            # tile is ready ~4us in and the rest hide behind the pipeline.
            yts = [ystat.tile([128, COLS], f16, name=f"yt{i_}")
                   for i_ in range(8)]
            for pt in yts:
                nc.vector.memset(pt[64:128, :], 0.0)

            for blk in range(N_BLK):
                yt = yts[blk % 8]
                if blk == 0:
                    q = COLS // 4
                    for pc in range(4):
                        nc.sync.dma_start(yt[0:64, pc * q:(pc + 1) * q],
                                          YB_d[0, 0:64, pc * q:(pc + 1) * q])
                    nc.sync.dma_start(yt[96:112, :], YB_d[0, 64:80, :])
                else:
                    nc.sync.dma_start(yt[0:64, :], YB_d[blk, 0:64, :])
                    nc.sync.dma_start(yt[96:112, :], YB_d[blk, 64:80, :])
                otile = opool.tile([64, COLS], f16)
                for ci in range(CPB):
                    cols = slice(CHUNK * ci, CHUNK * (ci + 1))
                    ps = pspool.tile([128, CHUNK], f32)
                    nc.tensor.matmul(ps[:], w[:], yt[:, cols],
                                     start=True, stop=True)   # K=80 fused y+b
                    if ci % 2 == 0:
                        nc.vector.tensor_copy(otile[:, cols], ps[0:64, :])
                    else:
                        nc.scalar.activation(otile[:, cols], ps[0:64, :],
                                             mybir.ActivationFunctionType.Copy)
                    if ci % 2 == 1:
                        # store every finished quarter-block: the output
                        # stream trickles steadily instead of bursting
                        # against the input stream
                        qs = slice(CHUNK * (ci - 1), CHUNK * (ci + 1))
                        nc.gpsimd.dma_start(Ot[blk, :, qs], otile[:, qs])
    nc.compile()  # bacc passes: split sync waits to HW limits, alloc regs, DCE
    return nc


def _get_program():
    if "nc" not in _prog_cache:
        _prog_cache["nc"] = _build_program()
    return _prog_cache["nc"]


def kernel(y, A, b):
    global last_results
    from concourse.bass_utils import run_bass_kernel_spmd

    y = np.ascontiguousarray(np.asarray(y, dtype=np.float32))
    b = np.ascontiguousarray(np.asarray(b, dtype=np.float32))
    A = np.asarray(A, dtype=np.float32)
    assert y.shape == (BATCH, IN_DIM) and b.shape == (BATCH, OUT_DIM)

    W = _build_weights(A)
    yh = y.astype(np.float16)
    bh = b.astype(np.float16)

    in_maps = []
    for core in range(N_CORES):
        sl = slice(core * SHARD, (core + 1) * SHARD)
        in_maps.append({"YB": _pack_in(yh[sl], bh[sl]), "W": W})

    nc = _get_program()
    res = run_bass_kernel_spmd(nc, in_maps, core_ids=list(range(N_CORES)))
    last_results = res

    out = np.empty((BATCH, IN_DIM), np.float32)
    for core in range(N_CORES):
        out[core * SHARD:(core + 1) * SHARD] = \
            _unpack_out(res.results[core]["Ot"]).astype(np.float32)
    return out


# revision 21
# speedup vs baseline: 1.3869x; 1.2643x over previous
"""Trainium2 Bass kernel for nn_ConstraintLayer (batched equality-constrained QP layer).

Math: the reference solves  M @ sol_i = [2*y_i; b_i]  for every batch row i,
with the SAME KKT matrix M = [[2I, A^T], [A, 0]] (80x80).  Since M is fixed,
    y_star = [2y, b] @ (M^{-1}[:64, :])^T  =  y @ Gy + b @ Gb
with Gy = 2*Minv[:64,:64].T (64x64) and Gb = Minv[:64,64:].T (16x64),
i.e. one skinny (batch,80)@(80,64) matmul - memory bound.

Distribution: pure data parallelism; the batch (1048576 rows) is split into 8
shards of 131072 rows, one per NeuronCore.  The tiny Gy/Gb factors are
precomputed once on host (float64 inverse of the 80x80 block matrix) and
replicated to every core.

Precision: the correctness gate is rel_err < 2e-2 (max-abs normalized), so a
single fp16 term per operand suffices: fp16(y) @ fp16(Gy) + fp16(b) @ fp16(Gb)
with fp32 PSUM accumulation, output stored fp16, measures 7.6e-4 against the
reference.  This halves HBM traffic vs an fp32-exact hi/lo-split kernel
(37MB vs 78MB per core) and the kernel is DMA-bound: the 16 SDMA engines
sustain ~19-20 GB/s each with the concurrent read+write streams, ~122us of
engine-busy per core, which is the wall.

Device layout (per core): the host interleaves y and b into ONE feature-major
stream: per 512-row chunk a [80, 512] tile (partitions 0:64 = y features,
64:80 = b features), 8 chunks per block -> [80, 4096] 640KB contiguous DMAs
(8KB per partition-line: the efficient DMA regime; 2KB/16KB lines measured
much slower).  Each chunk is a SINGLE K=80 matmul against the fused
stationary [Gy; Gb] (loaded once - no LDWEIGHTS swaps), M=64, N=512.

The PE runs at its throttled 1.2 GHz (427ns/matmul): the clock-release
(HAM) needs ~full-K matmuls (K=128 zero-padded variants DO run warm at
2.4 GHz / 213ns, but the freed PE time is unusable - the DMA wall paces the
kernel and the burstier consumption inflates DMA contention; measured slower
end-to-end).  At 427ns/MM the matmul train exactly matches the input stream
rate, which keeps every queue smooth.

PSUM eviction (f32 -> f16 cast) alternates between VectorE and the scalar
ACT engine; outputs leave as contiguous half-block (4KB-line) fp16 DMAs on
the SWDGE ring; the host inverts the packing.
"""

import numpy as np

BATCH = 1048576
IN_DIM = 64
OUT_DIM = 16
N_CORES = 8
SHARD = BATCH // N_CORES        # 131072
CHUNK = 512                     # batch rows per matmul (one PSUM bank col-span)
CPB = 8                         # chunks per block
N_BLK = SHARD // (CHUNK * CPB)  # 32
COLS = CHUNK * CPB              # 4096

_prog_cache = {}
last_results = None             # BassKernelResults of the most recent run (for test harness)


def _build_weights(A):
    """Host precompute of the fused stationary matrix (float64 inverse, fp16)."""
    m, n = A.shape  # (16, 64)
    A64 = np.asarray(A, dtype=np.float64)
    M = np.zeros((n + m, n + m))
    M[:n, :n] = 2.0 * np.eye(n)
    M[:n, n:] = A64.T
    M[n:, :n] = A64
    Minv = np.linalg.inv(M)
    W = np.empty((80, 64), np.float16)
    W[:64] = (2.0 * Minv[:n, :n].T).astype(np.float16)   # Gy
    W[64:] = (Minv[:n, n:].T).astype(np.float16)         # Gb
    return W


def _pack_in(ys, bs):
    # y (131072, 64) f16 + b (131072, 16) f16 -> blocks (32, 80, 4096);
    # partition = feature (y 0:64, b 64:80), col = 512*chunk_in_blk + s
    yb = ys.reshape(N_BLK, CPB, CHUNK, 64).transpose(0, 3, 1, 2).reshape(N_BLK, 64, COLS)
    bb = bs.reshape(N_BLK, CPB, CHUNK, 16).transpose(0, 3, 1, 2).reshape(N_BLK, 16, COLS)
    return np.ascontiguousarray(np.concatenate([yb, bb], axis=1))


def _unpack_out(ob):
    # (32, 64, 4096) f16 -> (131072, 64)
    return np.ascontiguousarray(
        ob.reshape(N_BLK, 64, CPB, CHUNK).transpose(0, 2, 3, 1)
    ).reshape(SHARD, 64)


def _build_program():
    import concourse.bacc as bacc
    import concourse.mybir as mybir
    import concourse.tile as tile

    f16 = mybir.dt.float16
    f32 = mybir.dt.float32
    nc = bacc.Bacc("TRN2")
    YB_d = nc.dram_tensor("YB", (N_BLK, 80, COLS), f16, kind="ExternalInput")
    W_d = nc.dram_tensor("W", (80, 64), f16, kind="ExternalInput")
    Ot = nc.dram_tensor("Ot", (N_BLK, 64, COLS), f16, kind="ExternalOutput")

    with tile.TileContext(nc) as tc:
        with (
            tc.tile_pool(name="wpool", bufs=1) as wpool,
            tc.tile_pool(name="ypool", bufs=6) as ypool,
            tc.tile_pool(name="opool", bufs=4) as opool,
            tc.tile_pool(name="pspool", bufs=8, space="PSUM") as pspool,
        ):
            # Inputs go through the sync-engine HWDGE ring, the (tiny) weight
            # through the scalar ring, outputs through SWDGE - independent
            # FIFOs so an output store never queues behind input traffic.
            # Block 0 is fetched in quarter-block pieces so the first matmul
            # only waits for the first 160KB instead of the whole 640KB.
            w = wpool.tile([80, 64], f16)
            nc.scalar.dma_start(w[:], W_d[:])

            for blk in range(N_BLK):
                yt = ypool.tile([80, COLS], f16)
                if blk == 0:
                    q = COLS // 4
                    for pc in range(4):
                        nc.sync.dma_start(yt[:, pc * q:(pc + 1) * q],
                                          YB_d[0, :, pc * q:(pc + 1) * q])
                else:
                    nc.sync.dma_start(yt[:], YB_d[blk])
                otile = opool.tile([64, COLS], f16)
                for ci in range(CPB):
                    cols = slice(CHUNK * ci, CHUNK * (ci + 1))
                    ps = pspool.tile([64, CHUNK], f32)
                    nc.tensor.matmul(ps[:], w[:], yt[:, cols],
                                     start=True, stop=True)   # K=80 fused y+b
                    if ci % 2 == 0:
                        nc.vector.tensor_copy(otile[:, cols], ps[:])
                    else:
                        nc.scalar.activation(otile[:, cols], ps[:],
                                             mybir.ActivationFunctionType.Copy)
                    if ci == CPB // 2 - 1:
                        # store the first half-block early: smooths the SWDGE
                        # queue and shortens the drain tail
                        nc.gpsimd.dma_start(Ot[blk, :, :COLS // 2],
                                            otile[:, :COLS // 2])
                nc.gpsimd.dma_start(Ot[blk, :, COLS // 2:], otile[:, COLS // 2:])
    nc.compile()  # bacc passes: split sync waits to HW limits, alloc regs, DCE
    return nc


def _get_program():
    if "nc" not in _prog_cache:
        _prog_cache["nc"] = _build_program()
    return _prog_cache["nc"]


def kernel(y, A, b):
    global last_results
    from concourse.bass_utils import run_bass_kernel_spmd

    y = np.ascontiguousarray(np.asarray(y, dtype=np.float32))
    b = np.ascontiguousarray(np.asarray(b, dtype=np.float32))
    A = np.asarray(A, dtype=np.float32)
    assert y.shape == (BATCH, IN_DIM) and b.shape == (BATCH, OUT_DIM)

    W = _build_weights(A)
    yh = y.astype(np.float16)
    bh = b.astype(np.float16)

    in_maps = []
    for core in range(N_CORES):
        sl = slice(core * SHARD, (core + 1) * SHARD)
        in_maps.append({"YB": _pack_in(yh[sl], bh[sl]), "W": W})

    nc = _get_program()
    res = run_bass_kernel_spmd(nc, in_maps, core_ids=list(range(N_CORES)))
    last_results = res

    out = np.empty((BATCH, IN_DIM), np.float32)
    for core in range(N_CORES):
        out[core * SHARD:(core + 1) * SHARD] = \
            _unpack_out(res.results[core]["Ot"]).astype(np.float32)
    return out
